# revision 13
# baseline (speedup 1.0000x reference)
"""Trainium2 Bass kernel for nn_Block_en_49469433315543 (involution block).

Computation (see reference):
  z = softplus(involution(x))          involution: per-pixel 3x3 dynamic kernel
  y = softplus(conv2d_3x3(z) + b_conv2)
with the per-pixel kernel = w_span @ relu(BN(w_reduce @ x)) + b_span, where BN
uses batch statistics over all 8 samples (requires a cross-core AllReduce).

Sharding: data-parallel over batch, one sample per NeuronCore (8 cores).
BN statistics via an augmented Gram matrix S = sum([x;1][x;1]^T) per core,
AllReduce'd early so the collective hides behind the r-matmul.

Layouts (per core, on-chip fp16 compute / fp32 psum+stats):
  x_cm16      [64, 16384]     channel-major (matmul rhs)
  xh_{m,0,p}  [128, 64|65, 132] h-on-partitions, free=(c, w+4 pad); the three
                              variants hold x[h-1], x[h], x[h+1] at partition h
                              (compute engines cannot partition-shift; DMA can,
                              so the host ships all three). xh_0 carries an
                              extra ones-channel for the Gram matmuls.
  kern_h      [128, 9, 128]   per-pixel kernel, h-on-partitions
  involution  acc[h, c, w] += kern_h[h, k, w] (bcast over c) * xh_dh[h, c, w+dw]
  z           DRAM round-trip to transpose h-major -> channel-major padded
  conv2       6 shift-pair matmuls (z stacked twice in partitions, +1 shifted)
"""
import sys

for _p in ("/opt/trn_rl_repo", "/root/.axon_site/_ro/trn_rl_repo"):
    if _p not in sys.path:
        sys.path.insert(0, _p)

import numpy as np

import concourse.bacc as bacc
import concourse.tile as tile
from concourse import mybir
from concourse.bass_utils import run_bass_kernel_spmd

C, H, W = 64, 128, 128
HW = H * W
N_CORES = 8
NTOT = N_CORES * HW
BN_EPS = 1e-5
WP = 132          # padded w width in h-major layout (2 zeros each side)
ZP = 130          # padded side of z in channel-major layout
F16 = mybir.dt.float16
F32 = mybir.dt.float32

_CACHE = {}


def _build():
    nc = bacc.Bacc()
    dp = nc.declare_dram_parameter
    x_cm = dp("x_cm", [C, HW], F16, isOutput=False)
    xh_m = dp("xh_m", [H, C, WP], F16, isOutput=False)
    xh_0 = dp("xh_0", [H, C + 1, WP], F16, isOutput=False)
    xh_p = dp("xh_p", [H, C, WP], F16, isOutput=False)
    wrT = dp("wrT", [C, C], F16, isOutput=False)       # w_reduce^T [c, o]
    wrow = dp("wrow", [C, C], F16, isOutput=False)     # w_reduce   [o, c]
    wspanT = dp("wspanT", [C, 9], F16, isOutput=False)
    bspan_bc = dp("bspan_bc", [H, 9], F32, isOutput=False)
    w_pair = [dp(f"wp{i}", [2 * C, C], F16, isOutput=False) for i in range(3)]
    w_sing = [dp(f"ws{i}", [C, C], F16, isOutput=False) for i in range(3)]
    gamma = dp("gamma", [C, 1], F32, isOutput=False)
    beta = dp("beta", [C, 1], F32, isOutput=False)
    bred = dp("bred", [C, 1], F32, isOutput=False)
    bconv = dp("bconv", [C, 1], F32, isOutput=False)
    y_out = dp("y", [C, HW], F32, isOutput=True)

    AF = mybir.ActivationFunctionType
    OP = mybir.AluOpType

    with tile.TileContext(nc) as tc:
        with (
            tc.tile_pool(name="sbuf", bufs=1) as pool,
            tc.tile_pool(name="rot", bufs=2) as rot,
            tc.tile_pool(name="psum", bufs=2, space="PSUM") as pp,
            tc.tile_pool(name="dram", bufs=1, space="DRAM") as dram,
        ):
            # ---- load inputs --------------------------------------------
            t_xcm = pool.tile([C, HW], F16)
            t_xhm = pool.tile([H, C, WP], F16)
            t_xh0 = pool.tile([H, C + 1, WP], F16)
            t_xhp = pool.tile([H, C, WP], F16)
            nc.sync.dma_start(t_xcm[:], x_cm[:])
            nc.sync.dma_start(t_xhm[:], xh_m[:])
            nc.sync.dma_start(t_xh0[:], xh_0[:])
            nc.sync.dma_start(t_xhp[:], xh_p[:])
            t_wrT = pool.tile([C, C], F16)
            t_wrow = pool.tile([C, C], F16)
            t_wspanT = pool.tile([C, 9], F16)
            t_bspan = pool.tile([H, 9], F32)
            nc.sync.dma_start(t_wrT[:], wrT[:])
            nc.sync.dma_start(t_wrow[:], wrow[:])
            nc.sync.dma_start(t_wspanT[:], wspanT[:])
            nc.sync.dma_start(t_bspan[:], bspan_bc[:])
            t_wp = [pool.tile([2 * C, C], F16, name=f"twp{i}", tag=f"wp{i}") for i in range(3)]
            t_ws = [pool.tile([C, C], F16, name=f"tws{i}", tag=f"ws{i}") for i in range(3)]
            for i in range(3):
                nc.sync.dma_start(t_wp[i][:], w_pair[i][:])
                nc.sync.dma_start(t_ws[i][:], w_sing[i][:])
            t_gamma = pool.tile([C, 1], F32)
            t_beta = pool.tile([C, 1], F32)
            t_bred = pool.tile([C, 1], F32)
            t_bconv = pool.tile([C, 1], F32)
            nc.sync.dma_start(t_gamma[:], gamma[:])
            nc.sync.dma_start(t_beta[:], beta[:])
            nc.sync.dma_start(t_bred[:], bred[:])
            nc.sync.dma_start(t_bconv[:], bconv[:])

            # ---- Gram stats: S_aug = sum over pixels of [x;1][x;1]^T ----
            ps_S = pp.tile([C + 1, C + 1], F32, tag="ps")
            for w in range(2, 2 + W):
                sl = t_xh0[:, 0 : C + 1, w]
                nc.tensor.matmul(
                    ps_S[:], lhsT=sl, rhs=sl, start=(w == 2), stop=(w == 2 + W - 1)
                )
            t_S32 = pool.tile([C + 1, C + 1], F32)
            nc.vector.tensor_copy(out=t_S32[:], in_=ps_S[:])

            # ---- AllReduce of S across the 8 cores ----------------------
            d_sin = dram.tile([C + 1, C + 1], F32)
            d_sout = dram.tile([C + 1, C + 1], F32)
            nc.sync.dma_start(d_sin[:], t_S32[:])
            nc.gpsimd.collective_compute(
                "AllReduce",
                OP.add,
                replica_groups=[list(range(N_CORES))],
                ins=[d_sin.opt()],
                outs=[d_sout.opt()],
            )
            t_G = pool.tile([C + 1, C + 1], F32)
            nc.sync.dma_start(t_G[:], d_sout[:])

            # ---- BN statistics from the Gram matrix ---------------------
            # xbar = G[0:64, 64] / N ; mu = W xbar + b
            t_xbar16 = pool.tile([C, 1], F16)
            nc.vector.tensor_scalar_mul(t_xbar16[:], t_G[0:C, C : C + 1], 1.0 / NTOT)
            ps_mu = pp.tile([C, 1], F32, tag="ps")
            nc.tensor.matmul(ps_mu[:], lhsT=t_wrT[:], rhs=t_xbar16[:])
            t_mu = pool.tile([C, 1], F32)
            nc.vector.tensor_tensor(out=t_mu[:], in0=ps_mu[:], in1=t_bred[:], op=OP.add)
            # T1 = W S ; diag = rowsum(T1 * W)
            t_S16 = pool.tile([C, C], F16)
            nc.vector.tensor_scalar_mul(t_S16[:], t_G[0:C, 0:C], 1.0 / NTOT)
            ps_T1 = pp.tile([C, C], F32, tag="ps")
            nc.tensor.matmul(ps_T1[:], lhsT=t_wrT[:], rhs=t_S16[:])
            t_q = pool.tile([C, C], F32)
            nc.vector.tensor_tensor(
                out=t_q[:], in0=ps_T1[:], in1=t_wrow[:], op=OP.mult
            )
            t_diag = pool.tile([C, 1], F32)
            nc.vector.tensor_reduce(
                t_diag[:], t_q[:], axis=mybir.AxisListType.X, op=OP.add
            )
            # E2 = diag/N + b*(2 mu - b); var = E2 - mu^2
            t_e2 = pool.tile([C, 1], F32)
            nc.vector.tensor_copy(out=t_e2[:], in_=t_diag[:])
            t_u = pool.tile([C, 1], F32)
            nc.vector.tensor_scalar_mul(t_u[:], t_mu[:], 2.0)
            nc.vector.tensor_tensor(out=t_u[:], in0=t_u[:], in1=t_bred[:], op=OP.subtract)
            nc.vector.tensor_tensor(out=t_u[:], in0=t_u[:], in1=t_bred[:], op=OP.mult)
            nc.vector.tensor_tensor(out=t_e2[:], in0=t_e2[:], in1=t_u[:], op=OP.add)
            t_mu2 = pool.tile([C, 1], F32)
            nc.vector.tensor_tensor(out=t_mu2[:], in0=t_mu[:], in1=t_mu[:], op=OP.mult)
            t_var = pool.tile([C, 1], F32)
            nc.vector.tensor_tensor(out=t_var[:], in0=t_e2[:], in1=t_mu2[:], op=OP.subtract)
            # rstd = exp(-0.5 ln(var + eps)); a = gamma*rstd; bb = a*(b-mu)+beta
            t_eps = pool.tile([C, 1], F32)
            nc.vector.memset(t_eps[:], BN_EPS)
            t_ln = pool.tile([C, 1], F32)
            nc.scalar.activation(t_ln[:], t_var[:], AF.Ln, bias=t_eps[:])
            t_rstd = pool.tile([C, 1], F32)
            nc.scalar.activation(t_rstd[:], t_ln[:], AF.Exp, scale=-0.5)
            t_a = pool.tile([C, 1], F32)
            nc.vector.tensor_tensor(out=t_a[:], in0=t_gamma[:], in1=t_rstd[:], op=OP.mult)
            t_bb = pool.tile([C, 1], F32)
            nc.vector.tensor_tensor(out=t_bb[:], in0=t_bred[:], in1=t_mu[:], op=OP.subtract)
            nc.vector.tensor_tensor(out=t_bb[:], in0=t_bb[:], in1=t_a[:], op=OP.mult)
            nc.vector.tensor_tensor(out=t_bb[:], in0=t_bb[:], in1=t_beta[:], op=OP.add)

            # ---- rn = relu(a * (W x) + bb), fused into the psum drain ---
            t_rn = pool.tile([C, HW], F16)
            RCH = 512
            for ch in range(HW // RCH):
                ps_r = pp.tile([C, RCH], F32, tag="ps")
                nc.tensor.matmul(
                    ps_r[:], lhsT=t_wrT[:], rhs=t_xcm[:, ch * RCH : (ch + 1) * RCH]
                )
                nc.scalar.activation(
                    t_rn[:, ch * RCH : (ch + 1) * RCH],
                    ps_r[:],
                    AF.Relu,
                    bias=t_bb[:],
                    scale=t_a[:],
                )

            # ---- per-pixel kernel, produced transposed into h-major -----
            # kern_h[h, k, w] = sum_c rn[c, h*128+w] wspanT[c, k] + b_span[k]
            t_kern = pool.tile([H, 9, W], F16)
            WG = 8
            for wg in range(W // WG):
                ps_k = pp.tile([H, 9 * WG], F32, tag="ps")
                rn_v = t_rn[:].rearrange("c (h w) -> c h w", h=H)
                for j in range(WG):
                    w = wg * WG + j
                    lhs = rn_v[:, :, w]  # [64, 128] stride W
                    nc.tensor.matmul(
                        ps_k[:, j * 9 : (j + 1) * 9], lhsT=lhs, rhs=t_wspanT[:]
                    )
                src = ps_k[:].rearrange("h (j k) -> h k j", k=9)
                dst = t_kern[:, :, wg * WG : (wg + 1) * WG]
                bias = t_bspan[:].rearrange("h (o k) -> h k o", o=1).broadcast_to(
                    [H, 9, WG]
                )
                nc.vector.scalar_tensor_tensor(
                    out=dst, in0=src, scalar=1.0, in1=bias, op0=OP.mult, op1=OP.add
                )

            # ---- involution MAC on DVE ----------------------------------
            t_acc = pool.tile([H, C, W], F16)
            xh_by_dh = {-1: t_xhm, 0: t_xh0, 1: t_xhp}
            first = True
            for i in range(3):
                for j in range(3):
                    k = i * 3 + j
                    dh, dw = i - 1, j - 1
                    xt = xh_by_dh[dh]
                    x_sl = xt[:, 0:C, 2 + dw : 2 + dw + W]
                    k_bc = (
                        t_kern[:, k, :]
                        .rearrange("h (o w) -> h o w", o=1)
                        .broadcast_to([H, C, W])
                    )
                    if first:
                        nc.vector.tensor_tensor(
                            out=t_acc[:], in0=x_sl, in1=k_bc, op=OP.mult
                        )
                        first = False
                    else:
                        t_tmp = rot.tile([H, C, W], F16, tag="mactmp")
                        nc.vector.tensor_tensor(
                            out=t_tmp[:], in0=x_sl, in1=k_bc, op=OP.mult
                        )
                        nc.vector.tensor_tensor(
                            out=t_acc[:], in0=t_acc[:], in1=t_tmp[:], op=OP.add
                        )

            # ---- softplus(acc) = ln(1 + exp(acc)), pipelined by w-half --
            # and z transpose (h-major -> cm padded) via DRAM, in w-halves
            # on three different HWDGE queues so write/read overlap.
            t_e = pool.tile([H, C, W], F32, tag="t_xcm")
            t_zh = pool.tile([H, C, W], F16, tag="t_xhm")
            d_z = dram.tile([C, ZP * ZP], F16)
            t_zz = pool.tile([2 * C, ZP * ZP], F16, tag="t_rn")
            nc.gpsimd.memset(t_zz[:], 0.0)
            dz_w = d_z[:].rearrange("c (h w) -> h c w", h=ZP)
            dz_r = d_z[:].rearrange("c (h w) -> c h w", h=ZP)
            zz_lo_v = t_zz[0:C, :].rearrange("c (h w) -> c h w", h=ZP)
            zz_hi_v = t_zz[C : 2 * C, :].rearrange("c (h w) -> c h w", h=ZP)
            WHALF = W // 2
            for a in (0, WHALF):
                b = a + WHALF
                nc.scalar.activation(t_e[:, :, a:b], t_acc[:, :, a:b], AF.Exp)
                nc.scalar.activation(t_zh[:, :, a:b], t_e[:, :, a:b], AF.Ln, bias=1.0)
                nc.sync.dma_start(dz_w[1 : 1 + H, :, 1 + a : 1 + b], t_zh[:, :, a:b])
                src_s = dz_r[:, 1 : 1 + H, 1 + a : 1 + b]
                nc.scalar.dma_start(zz_lo_v[:, 1 : 1 + H, 1 + a : 1 + b], src_s)
                nc.gpsimd.dma_start(zz_hi_v[:, 1 : 1 + H, a:b], src_s)

            # ---- conv2 (6 matmuls per chunk) + softplus + store ---------
            CROWS = 16  # output rows per psum chunk
            NSUB = CROWS // 4
            for ch in range(H // CROWS):
                ps_y = pp.tile([C, CROWS * W], F32, tag="ps")
                for sub in range(NSUB):
                    h0 = ch * CROWS + sub * 4
                    zz_v2 = t_zz[:].rearrange("c (h w) -> c h w", h=ZP)
                    for t in range(6):
                        if t < 3:
                            i = t
                            lhsT_w = t_wp[i][:]
                            part = 2 * C
                            c0 = 0
                        else:
                            i = t - 3
                            lhsT_w = t_ws[i][:]
                            part = C
                            c0 = 2
                        r0 = h0 + i
                        src2 = zz_v2[0:part, r0 : r0 + 4, c0 : c0 + W]
                        nc.tensor.matmul(
                            ps_y[:, sub * 512 : (sub + 1) * 512],
                            lhsT=lhsT_w,
                            rhs=src2,
                            start=(t == 0),
                            stop=(t == 5),
                        )
                t_ey = rot.tile([C, CROWS * W], F32, tag="ey")
                nc.scalar.activation(t_ey[:], ps_y[:], AF.Exp, bias=t_bconv[:])
                t_y = rot.tile([C, CROWS * W], F32, tag="yc")
                nc.scalar.activation(t_y[:], t_ey[:], AF.Ln, bias=1.0)
                nc.sync.dma_start(
                    y_out[:, ch * CROWS * W : (ch + 1) * CROWS * W], t_y[:]
                )

    nc.compile()
    return nc


def _prep_core_inputs(xs, w_reduce, b_reduce, bn_gamma, bn_beta, w_span, b_span,
                      w_conv2, b_conv2):
    """Host-side layout prep for one core's sample xs [C, H, W] fp32."""
    xhw = xs.transpose(1, 0, 2)  # [h, c, w]
    xh_0 = np.zeros((H, C + 1, WP), np.float16)
    xh_0[:, 0:C, 2 : 2 + W] = xhw
    xh_0[:, C, 2 : 2 + W] = 1.0
    xh_m = np.zeros((H, C, WP), np.float16)
    xh_m[1:H, :, 2 : 2 + W] = xhw[0 : H - 1]
    xh_p = np.zeros((H, C, WP), np.float16)
    xh_p[0 : H - 1, :, 2 : 2 + W] = xhw[1:H]
    w_pair = []
    w_sing = []
    for i in range(3):
        wp = np.concatenate(
            [w_conv2[:, :, i, 0].T, w_conv2[:, :, i, 1].T], axis=0
        ).astype(np.float16)
        w_pair.append(np.ascontiguousarray(wp))
        w_sing.append(np.ascontiguousarray(w_conv2[:, :, i, 2].T).astype(np.float16))
    m = {
        "x_cm": xs.reshape(C, HW).astype(np.float16),
        "xh_m": xh_m,
        "xh_0": xh_0,
        "xh_p": xh_p,
        "wrT": np.ascontiguousarray(w_reduce.T).astype(np.float16),
        "wrow": np.ascontiguousarray(w_reduce).astype(np.float16),
        "wspanT": np.ascontiguousarray(w_span.T).astype(np.float16),
        "bspan_bc": np.tile(b_span.astype(np.float32)[None, :], (H, 1)),
        "gamma": bn_gamma.astype(np.float32).reshape(C, 1),
        "beta": bn_beta.astype(np.float32).reshape(C, 1),
        "bred": b_reduce.astype(np.float32).reshape(C, 1),
        "bconv": b_conv2.astype(np.float32).reshape(C, 1),
    }
    for i in range(3):
        m[f"wp{i}"] = w_pair[i]
        m[f"ws{i}"] = w_sing[i]
    return m


def kernel(x, w_reduce, b_reduce, bn_gamma, bn_beta, w_span, b_span, w_conv2,
           b_conv2):
    x = np.asarray(x, np.float32)
    if "nc" not in _CACHE:
        _CACHE["nc"] = _build()
    nc = _CACHE["nc"]
    in_maps = [
        _prep_core_inputs(
            x[b], np.asarray(w_reduce, np.float32), np.asarray(b_reduce, np.float32),
            np.asarray(bn_gamma, np.float32), np.asarray(bn_beta, np.float32),
            np.asarray(w_span, np.float32), np.asarray(b_span, np.float32),
            np.asarray(w_conv2, np.float32), np.asarray(b_conv2, np.float32),
        )
        for b in range(N_CORES)
    ]
    res = run_bass_kernel_spmd(nc, in_maps, core_ids=list(range(N_CORES)))
    out = np.stack([res.results[b]["y"].reshape(C, H, W) for b in range(N_CORES)])
    return out.astype(np.float32)


# revision 14
# speedup vs baseline: 1.1675x; 1.1675x over previous
"""Trainium2 Bass kernel for nn_Block_en_49469433315543 (involution block).

Computation (see reference):
  z = softplus(involution(x))          involution: per-pixel 3x3 dynamic kernel
  y = softplus(conv2d_3x3(z) + b_conv2)
with the per-pixel kernel = w_span @ relu(BN(w_reduce @ x)) + b_span, where BN
uses batch statistics over all 8 samples (requires a cross-core AllReduce).

Sharding: data-parallel over batch, one sample per NeuronCore (8 cores).
BN statistics via an augmented Gram matrix S = sum([x;1][x;1]^T) per core,
AllReduce'd early so the collective hides behind the r-matmul.

Layouts (per core, on-chip fp16 compute / fp32 psum+stats):
  x_cm16      [64, 16384]     channel-major (matmul rhs)
  xh_{m,0,p}  [128, 64|65, 132] h-on-partitions, free=(c, w+4 pad); the three
                              variants hold x[h-1], x[h], x[h+1] at partition h
                              (compute engines cannot partition-shift; DMA can,
                              so the host ships all three). xh_0 carries an
                              extra ones-channel for the Gram matmuls.
  kern_h      [128, 9, 128]   per-pixel kernel, h-on-partitions
  involution  acc[h, c, w] += kern_h[h, k, w] (bcast over c) * xh_dh[h, c, w+dw]
  z           DRAM round-trip to transpose h-major -> channel-major padded
  conv2       6 shift-pair matmuls (z stacked twice in partitions, +1 shifted)
"""
import sys

for _p in ("/opt/trn_rl_repo", "/root/.axon_site/_ro/trn_rl_repo"):
    if _p not in sys.path:
        sys.path.insert(0, _p)

import numpy as np

import concourse.bacc as bacc
import concourse.tile as tile
from concourse import mybir
from concourse.bass_utils import run_bass_kernel_spmd

C, H, W = 64, 128, 128
HW = H * W
N_CORES = 8
NTOT = N_CORES * HW
BN_EPS = 1e-5
WP = 132          # padded w width in h-major layout (2 zeros each side)
ZP = 130          # padded side of z in channel-major layout
F16 = mybir.dt.float16
F32 = mybir.dt.float32

_CACHE = {}


def _build():
    nc = bacc.Bacc()
    dp = nc.declare_dram_parameter
    x_cm = dp("x_cm", [C, HW], F16, isOutput=False)
    xh_m = dp("xh_m", [H, C, WP], F16, isOutput=False)
    xh_0 = dp("xh_0", [H, C + 1, WP], F16, isOutput=False)
    xh_p = dp("xh_p", [H, C, WP], F16, isOutput=False)
    wrT = dp("wrT", [C, C], F16, isOutput=False)       # w_reduce^T [c, o]
    wrow = dp("wrow", [C, C], F16, isOutput=False)     # w_reduce   [o, c]
    wspanT = dp("wspanT", [C, 9], F16, isOutput=False)
    bspan_bc = dp("bspan_bc", [H, 9], F32, isOutput=False)
    w_pair = [dp(f"wp{i}", [2 * C, C], F16, isOutput=False) for i in range(3)]
    w_sing = [dp(f"ws{i}", [C, C], F16, isOutput=False) for i in range(3)]
    gamma = dp("gamma", [C, 1], F32, isOutput=False)
    beta = dp("beta", [C, 1], F32, isOutput=False)
    bred = dp("bred", [C, 1], F32, isOutput=False)
    bconv = dp("bconv", [C, 1], F32, isOutput=False)
    y_out = dp("y", [C, HW], F32, isOutput=True)

    AF = mybir.ActivationFunctionType
    OP = mybir.AluOpType

    with tile.TileContext(nc) as tc:
        with (
            tc.tile_pool(name="sbuf", bufs=1) as pool,
            tc.tile_pool(name="rot", bufs=2) as rot,
            tc.tile_pool(name="psum", bufs=2, space="PSUM") as pp,
            tc.tile_pool(name="dram", bufs=1, space="DRAM") as dram,
        ):
            # ---- load inputs --------------------------------------------
            t_xcm = pool.tile([C, HW], F16)
            t_xhm = pool.tile([H, C, WP], F16)
            t_xh0 = pool.tile([H, C + 1, WP], F16)
            t_xhp = pool.tile([H, C, WP], F16)
            nc.sync.dma_start(t_xcm[:], x_cm[:])
            nc.sync.dma_start(t_xhm[:], xh_m[:])
            nc.sync.dma_start(t_xh0[:], xh_0[:])
            nc.sync.dma_start(t_xhp[:], xh_p[:])
            t_wrT = pool.tile([C, C], F16)
            t_wrow = pool.tile([C, C], F16)
            t_wspanT = pool.tile([C, 9], F16)
            t_bspan = pool.tile([H, 9], F32)
            nc.sync.dma_start(t_wrT[:], wrT[:])
            nc.sync.dma_start(t_wrow[:], wrow[:])
            nc.sync.dma_start(t_wspanT[:], wspanT[:])
            nc.sync.dma_start(t_bspan[:], bspan_bc[:])
            t_wp = [pool.tile([2 * C, C], F16, name=f"twp{i}", tag=f"wp{i}") for i in range(3)]
            t_ws = [pool.tile([C, C], F16, name=f"tws{i}", tag=f"ws{i}") for i in range(3)]
            for i in range(3):
                nc.sync.dma_start(t_wp[i][:], w_pair[i][:])
                nc.sync.dma_start(t_ws[i][:], w_sing[i][:])
            t_gamma = pool.tile([C, 1], F32)
            t_beta = pool.tile([C, 1], F32)
            t_bred = pool.tile([C, 1], F32)
            t_bconv = pool.tile([C, 1], F32)
            nc.sync.dma_start(t_gamma[:], gamma[:])
            nc.sync.dma_start(t_beta[:], beta[:])
            nc.sync.dma_start(t_bred[:], bred[:])
            nc.sync.dma_start(t_bconv[:], bconv[:])

            # ---- Gram stats: S_aug = sum over pixels of [x;1][x;1]^T ----
            ps_S = pp.tile([C + 1, C + 1], F32, tag="ps")
            for w in range(2, 2 + W):
                sl = t_xh0[:, 0 : C + 1, w]
                nc.tensor.matmul(
                    ps_S[:], lhsT=sl, rhs=sl, start=(w == 2), stop=(w == 2 + W - 1)
                )
            t_S32 = pool.tile([C + 1, C + 1], F32)
            nc.vector.tensor_copy(out=t_S32[:], in_=ps_S[:])

            # ---- AllReduce of S across the 8 cores ----------------------
            d_sin = dram.tile([C + 1, C + 1], F32)
            d_sout = dram.tile([C + 1, C + 1], F32)
            nc.sync.dma_start(d_sin[:], t_S32[:])
            nc.gpsimd.collective_compute(
                "AllReduce",
                OP.add,
                replica_groups=[list(range(N_CORES))],
                ins=[d_sin.opt()],
                outs=[d_sout.opt()],
            )
            t_G = pool.tile([C + 1, C + 1], F32)
            nc.sync.dma_start(t_G[:], d_sout[:])

            # ---- BN statistics from the Gram matrix ---------------------
            # xbar = G[0:64, 64] / N ; mu = W xbar + b
            t_xbar16 = pool.tile([C, 1], F16)
            nc.vector.tensor_scalar_mul(t_xbar16[:], t_G[0:C, C : C + 1], 1.0 / NTOT)
            ps_mu = pp.tile([C, 1], F32, tag="ps")
            nc.tensor.matmul(ps_mu[:], lhsT=t_wrT[:], rhs=t_xbar16[:])
            t_mu = pool.tile([C, 1], F32)
            nc.vector.tensor_tensor(out=t_mu[:], in0=ps_mu[:], in1=t_bred[:], op=OP.add)
            # T1 = W S ; diag = rowsum(T1 * W)
            t_S16 = pool.tile([C, C], F16)
            nc.vector.tensor_scalar_mul(t_S16[:], t_G[0:C, 0:C], 1.0 / NTOT)
            ps_T1 = pp.tile([C, C], F32, tag="ps")
            nc.tensor.matmul(ps_T1[:], lhsT=t_wrT[:], rhs=t_S16[:])
            t_q = pool.tile([C, C], F32)
            nc.vector.tensor_tensor(
                out=t_q[:], in0=ps_T1[:], in1=t_wrow[:], op=OP.mult
            )
            t_diag = pool.tile([C, 1], F32)
            nc.vector.tensor_reduce(
                t_diag[:], t_q[:], axis=mybir.AxisListType.X, op=OP.add
            )
            # E2 = diag/N + b*(2 mu - b); var = E2 - mu^2
            t_e2 = pool.tile([C, 1], F32)
            nc.vector.tensor_copy(out=t_e2[:], in_=t_diag[:])
            t_u = pool.tile([C, 1], F32)
            nc.vector.tensor_scalar_mul(t_u[:], t_mu[:], 2.0)
            nc.vector.tensor_tensor(out=t_u[:], in0=t_u[:], in1=t_bred[:], op=OP.subtract)
            nc.vector.tensor_tensor(out=t_u[:], in0=t_u[:], in1=t_bred[:], op=OP.mult)
            nc.vector.tensor_tensor(out=t_e2[:], in0=t_e2[:], in1=t_u[:], op=OP.add)
            t_mu2 = pool.tile([C, 1], F32)
            nc.vector.tensor_tensor(out=t_mu2[:], in0=t_mu[:], in1=t_mu[:], op=OP.mult)
            t_var = pool.tile([C, 1], F32)
            nc.vector.tensor_tensor(out=t_var[:], in0=t_e2[:], in1=t_mu2[:], op=OP.subtract)
            # rstd = exp(-0.5 ln(var + eps)); a = gamma*rstd; bb = a*(b-mu)+beta
            t_eps = pool.tile([C, 1], F32)
            nc.vector.memset(t_eps[:], BN_EPS)
            t_ln = pool.tile([C, 1], F32)
            nc.scalar.activation(t_ln[:], t_var[:], AF.Ln, bias=t_eps[:])
            t_rstd = pool.tile([C, 1], F32)
            nc.scalar.activation(t_rstd[:], t_ln[:], AF.Exp, scale=-0.5)
            t_a = pool.tile([C, 1], F32)
            nc.vector.tensor_tensor(out=t_a[:], in0=t_gamma[:], in1=t_rstd[:], op=OP.mult)
            t_bb = pool.tile([C, 1], F32)
            nc.vector.tensor_tensor(out=t_bb[:], in0=t_bred[:], in1=t_mu[:], op=OP.subtract)
            nc.vector.tensor_tensor(out=t_bb[:], in0=t_bb[:], in1=t_a[:], op=OP.mult)
            nc.vector.tensor_tensor(out=t_bb[:], in0=t_bb[:], in1=t_beta[:], op=OP.add)

            # ---- rn = relu(a * (W x) + bb), fused into the psum drain ---
            t_rn = pool.tile([C, HW], F16)
            RCH = 512
            for ch in range(HW // RCH):
                ps_r = pp.tile([C, RCH], F32, tag="ps")
                nc.tensor.matmul(
                    ps_r[:], lhsT=t_wrT[:], rhs=t_xcm[:, ch * RCH : (ch + 1) * RCH]
                )
                nc.scalar.activation(
                    t_rn[:, ch * RCH : (ch + 1) * RCH],
                    ps_r[:],
                    AF.Relu,
                    bias=t_bb[:],
                    scale=t_a[:],
                )

            # ---- per-pixel kernel, produced transposed into h-major -----
            # kern_h[h, k, w] = sum_c rn[c, h*128+w] wspanT[c, k] + b_span[k]
            t_kern = pool.tile([H, 9, W], F16)
            WG = 8
            for wg in range(W // WG):
                ps_k = pp.tile([H, 9 * WG], F32, tag="ps")
                rn_v = t_rn[:].rearrange("c (h w) -> c h w", h=H)
                for j in range(WG):
                    w = wg * WG + j
                    lhs = rn_v[:, :, w]  # [64, 128] stride W
                    nc.tensor.matmul(
                        ps_k[:, j * 9 : (j + 1) * 9], lhsT=lhs, rhs=t_wspanT[:]
                    )
                src = ps_k[:].rearrange("h (j k) -> h k j", k=9)
                dst = t_kern[:, :, wg * WG : (wg + 1) * WG]
                bias = t_bspan[:].rearrange("h (o k) -> h k o", o=1).broadcast_to(
                    [H, 9, WG]
                )
                nc.vector.scalar_tensor_tensor(
                    out=dst, in0=src, scalar=1.0, in1=bias, op0=OP.mult, op1=OP.add
                )

            # ---- involution MAC on DVE ----------------------------------
            t_acc = pool.tile([H, C, W], F16)
            xh_by_dh = {-1: t_xhm, 0: t_xh0, 1: t_xhp}
            first = True
            for i in range(3):
                for j in range(3):
                    k = i * 3 + j
                    dh, dw = i - 1, j - 1
                    xt = xh_by_dh[dh]
                    x_sl = xt[:, 0:C, 2 + dw : 2 + dw + W]
                    k_bc = (
                        t_kern[:, k, :]
                        .rearrange("h (o w) -> h o w", o=1)
                        .broadcast_to([H, C, W])
                    )
                    if first:
                        nc.vector.tensor_tensor(
                            out=t_acc[:], in0=x_sl, in1=k_bc, op=OP.mult
                        )
                        first = False
                    else:
                        t_tmp = rot.tile([H, C, W], F16, tag="mactmp")
                        nc.vector.tensor_tensor(
                            out=t_tmp[:], in0=x_sl, in1=k_bc, op=OP.mult
                        )
                        nc.vector.tensor_tensor(
                            out=t_acc[:], in0=t_acc[:], in1=t_tmp[:], op=OP.add
                        )

            # ---- softplus(acc) = ln(1 + exp(acc)) -----------------------
            t_e = pool.tile([H, C, W], F32, tag="t_xcm")
            nc.scalar.activation(t_e[:], t_acc[:], AF.Exp)
            t_zh = pool.tile([H, C, W], F16, tag="t_xhm")
            nc.scalar.activation(t_zh[:], t_e[:], AF.Ln, bias=1.0)

            # ---- z: h-major -> cm padded via DRAM; reads on two queues --
            d_z = dram.tile([C, ZP * ZP], F16)
            dz_v = d_z[:].rearrange("c (h w) -> h c w", h=ZP)[1 : 1 + H, :, 1 : 1 + W]
            nc.sync.dma_start(dz_v, t_zh[:])
            t_zz = pool.tile([2 * C, ZP * ZP], F16, tag="t_rn")
            nc.gpsimd.memset(t_zz[:], 0.0)
            zz_lo = t_zz[0:C, :].rearrange("c (h w) -> c h w", h=ZP)[
                :, 1 : 1 + H, 1 : 1 + W
            ]
            zz_hi = t_zz[C : 2 * C, :].rearrange("c (h w) -> c h w", h=ZP)[
                :, 1 : 1 + H, 0:W
            ]
            d_src = d_z[:].rearrange("c (h w) -> c h w", h=ZP)[:, 1 : 1 + H, 1 : 1 + W]
            nc.scalar.dma_start(zz_lo, d_src)
            nc.sync.dma_start(zz_hi, d_src)

            # ---- conv2 (6 matmuls per chunk) + softplus + store ---------
            CROWS = 16  # output rows per psum chunk
            NSUB = CROWS // 4
            for ch in range(H // CROWS):
                ps_y = pp.tile([C, CROWS * W], F32, tag="ps")
                for sub in range(NSUB):
                    h0 = ch * CROWS + sub * 4
                    zz_v2 = t_zz[:].rearrange("c (h w) -> c h w", h=ZP)
                    for t in range(6):
                        if t < 3:
                            i = t
                            lhsT_w = t_wp[i][:]
                            part = 2 * C
                            c0 = 0
                        else:
                            i = t - 3
                            lhsT_w = t_ws[i][:]
                            part = C
                            c0 = 2
                        r0 = h0 + i
                        src2 = zz_v2[0:part, r0 : r0 + 4, c0 : c0 + W]
                        nc.tensor.matmul(
                            ps_y[:, sub * 512 : (sub + 1) * 512],
                            lhsT=lhsT_w,
                            rhs=src2,
                            start=(t == 0),
                            stop=(t == 5),
                        )
                t_ey = rot.tile([C, CROWS * W], F32, tag="ey")
                nc.scalar.activation(t_ey[:], ps_y[:], AF.Exp, bias=t_bconv[:])
                t_y = rot.tile([C, CROWS * W], F32, tag="yc")
                nc.scalar.activation(t_y[:], t_ey[:], AF.Ln, bias=1.0)
                nc.sync.dma_start(
                    y_out[:, ch * CROWS * W : (ch + 1) * CROWS * W], t_y[:]
                )

    nc.compile()
    return nc


def _prep_core_inputs(xs, w_reduce, b_reduce, bn_gamma, bn_beta, w_span, b_span,
                      w_conv2, b_conv2):
    """Host-side layout prep for one core's sample xs [C, H, W] fp32."""
    xhw = xs.transpose(1, 0, 2)  # [h, c, w]
    xh_0 = np.zeros((H, C + 1, WP), np.float16)
    xh_0[:, 0:C, 2 : 2 + W] = xhw
    xh_0[:, C, 2 : 2 + W] = 1.0
    xh_m = np.zeros((H, C, WP), np.float16)
    xh_m[1:H, :, 2 : 2 + W] = xhw[0 : H - 1]
    xh_p = np.zeros((H, C, WP), np.float16)
    xh_p[0 : H - 1, :, 2 : 2 + W] = xhw[1:H]
    w_pair = []
    w_sing = []
    for i in range(3):
        wp = np.concatenate(
            [w_conv2[:, :, i, 0].T, w_conv2[:, :, i, 1].T], axis=0
        ).astype(np.float16)
        w_pair.append(np.ascontiguousarray(wp))
        w_sing.append(np.ascontiguousarray(w_conv2[:, :, i, 2].T).astype(np.float16))
    m = {
        "x_cm": xs.reshape(C, HW).astype(np.float16),
        "xh_m": xh_m,
        "xh_0": xh_0,
        "xh_p": xh_p,
        "wrT": np.ascontiguousarray(w_reduce.T).astype(np.float16),
        "wrow": np.ascontiguousarray(w_reduce).astype(np.float16),
        "wspanT": np.ascontiguousarray(w_span.T).astype(np.float16),
        "bspan_bc": np.tile(b_span.astype(np.float32)[None, :], (H, 1)),
        "gamma": bn_gamma.astype(np.float32).reshape(C, 1),
        "beta": bn_beta.astype(np.float32).reshape(C, 1),
        "bred": b_reduce.astype(np.float32).reshape(C, 1),
        "bconv": b_conv2.astype(np.float32).reshape(C, 1),
    }
    for i in range(3):
        m[f"wp{i}"] = w_pair[i]
        m[f"ws{i}"] = w_sing[i]
    return m


def kernel(x, w_reduce, b_reduce, bn_gamma, bn_beta, w_span, b_span, w_conv2,
           b_conv2):
    x = np.asarray(x, np.float32)
    if "nc" not in _CACHE:
        _CACHE["nc"] = _build()
    nc = _CACHE["nc"]
    in_maps = [
        _prep_core_inputs(
            x[b], np.asarray(w_reduce, np.float32), np.asarray(b_reduce, np.float32),
            np.asarray(bn_gamma, np.float32), np.asarray(bn_beta, np.float32),
            np.asarray(w_span, np.float32), np.asarray(b_span, np.float32),
            np.asarray(w_conv2, np.float32), np.asarray(b_conv2, np.float32),
        )
        for b in range(N_CORES)
    ]
    res = run_bass_kernel_spmd(nc, in_maps, core_ids=list(range(N_CORES)))
    out = np.stack([res.results[b]["y"].reshape(C, H, W) for b in range(N_CORES)])
    return out.astype(np.float32)
